# revision 3
# baseline (speedup 1.0000x reference)
"""Tensor-parallel (F-sharded) MoE MLP (Llama4 text experts) for 8 Trainium2 cores.

Strategy: shard the intermediate dimension F across the 8 cores. Core c holds,
for ALL 8 experts, gate columns [c*FL,(c+1)*FL) and up columns [F+c*FL, ...) of
gate_up_proj, plus rows [c*FL,(c+1)*FL) of down_proj (FL = F/8 = 256). Every
core processes ALL tokens (sorted by expert id): for each expert segment it
computes h = up * silu(gate) over its 256-wide F slice and a partial
y_c = h @ Wd_slice; the host sums the 8 partials. Segment widths are the global
expert counts - identical on every core - so one uniform SPMD program covers
all cores with zero token padding and perfect load balance.

Everything runs in the transposed layout (weights stationary, tokens stream):
h^T = up^T * silu(Wgu^T x^T), y^T = Wd^T h^T. bf16 in, fp32 PSUM, bf16 partial
out (host accumulates in fp32). The program is specialized to the expert-count
tuple (host knows counts before launch; compiled kernels are cached).
"""

import numpy as np
import ml_dtypes

_BF16 = ml_dtypes.bfloat16
_NC = 8           # cores
_T = 8192         # tokens
_H = 1024         # hidden
_F = 2048         # intermediate
_FL = _F // _NC   # 256 local F columns per core
_F2L = 2 * _FL    # 512 = [gate | up] local columns

_nc_cache: dict = {}
last_run = None  # BassKernelResults of the most recent kernel() call


def _chunks_of(n, cap=512):
    """Split n columns into near-equal chunks of at most cap."""
    if n <= 0:
        return []
    npc = -(-n // cap)
    base, rem = divmod(n, npc)
    sizes = [base + 1] * rem + [base] * (npc - rem)
    out, off = [], 0
    for s in sizes:
        out.append((off, s))
        off += s
    return out


def _build(counts):
    import concourse.bacc as bacc
    import concourse.mybir as mybir
    from concourse.tile import TileContext

    nc = bacc.Bacc()
    KB1 = _H // 128   # contraction chunks for x @ Wgu
    KB2 = _FL // 128  # contraction chunks for h @ Wd (= 2)
    NH = _H // 128    # output row tiles of y

    xT = nc.dram_tensor("xT", [_H, _T], mybir.dt.bfloat16, kind="ExternalInput")
    wgu = nc.dram_tensor(
        "wgu", [_NC * _H, _F2L], mybir.dt.bfloat16, kind="ExternalInput"
    )
    wd = nc.dram_tensor("wd", [_NC * _FL, _H], mybir.dt.bfloat16, kind="ExternalInput")
    yT = nc.dram_tensor("yT", [_H, _T], mybir.dt.bfloat16, kind="ExternalOutput")

    segs = []
    t0 = 0
    for e in range(_NC):
        n = int(counts[e])
        if n:
            segs.append((e, t0, _chunks_of(n)))
        t0 += n
    max_c = max(int(c) for c in counts)

    with TileContext(nc) as tc:
        with (
            tc.tile_pool(name="wgu_p", bufs=1) as wgu_p,
            tc.tile_pool(name="wd_p", bufs=1) as wd_p,
            tc.tile_pool(name="x_p", bufs=3) as x_p,
            tc.tile_pool(name="silu_p", bufs=3) as silu_p,
            tc.tile_pool(name="up_p", bufs=3) as up_p,
            tc.tile_pool(name="h_p", bufs=2) as h_p,
            tc.tile_pool(name="y_p", bufs=8) as y_p,
            tc.tile_pool(name="ps1", bufs=4, space="PSUM") as ps1_p,
            tc.tile_pool(name="ps2", bufs=3, space="PSUM") as ps2_p,
        ):
            wgu_sb: dict = {}
            wd_sb: dict = {}
            x_sb: dict = {}

            def issue_w(e):
                if e in wgu_sb:
                    return
                gt = [
                    wgu_p.tile([128, _F2L], mybir.dt.bfloat16, name=f"wgu{e}_{k}", tag=f"wgu{e}_{k}")
                    for k in range(KB1)
                ]
                for k in range(KB1):
                    nc.sync.dma_start(
                        out=gt[k], in_=wgu[e * _H + k * 128 : e * _H + (k + 1) * 128, :]
                    )
                wgu_sb[e] = gt
                dts = [
                    wd_p.tile([128, _H], mybir.dt.bfloat16, name=f"wd{e}_{k}", tag=f"wd{e}_{k}")
                    for k in range(KB2)
                ]
                for k in range(KB2):
                    nc.sync.dma_start(
                        out=dts[k],
                        in_=wd[e * _FL + k * 128 : e * _FL + (k + 1) * 128, :],
                    )
                wd_sb[e] = dts

            def issue_x(si):
                e, s0, chs = segs[si]
                n = chs[-1][0] + chs[-1][1]
                ts = [
                    x_p.tile([128, max_c], mybir.dt.bfloat16, name=f"x{si}_{k}", tag=f"x{k}")
                    for k in range(KB1)
                ]
                for k in range(KB1):
                    nc.sync.dma_start(
                        out=ts[k][:, :n], in_=xT[k * 128 : (k + 1) * 128, s0 : s0 + n]
                    )
                x_sb[si] = ts

            nseg = len(segs)
            issue_x(0)
            issue_w(segs[0][0])
            if nseg > 1:
                issue_x(1)
                issue_w(segs[1][0])

            for si, (e, s0, chs) in enumerate(segs):
                xs = x_sb[si]
                first = True
                for off, nb in chs:
                    if first and si + 2 < nseg:
                        issue_x(si + 2)
                        issue_w(segs[si + 2][0])
                    first = False
                    h_tiles = []
                    for i in range(2):  # two 128-row f-local tiles
                        ps_g = ps1_p.tile([128, 512], mybir.dt.float32, name="ps1", tag="ps1")
                        for k in range(KB1):
                            nc.tensor.matmul(
                                out=ps_g[:, :nb],
                                lhsT=wgu_sb[e][k][:, i * 128 : (i + 1) * 128],
                                rhs=xs[k][:, off : off + nb],
                                start=(k == 0),
                                stop=(k == KB1 - 1),
                            )
                        st = silu_p.tile([128, 512], mybir.dt.bfloat16, name="silu", tag="silu")
                        nc.scalar.activation(
                            st[:, :nb], ps_g[:, :nb], mybir.ActivationFunctionType.Silu
                        )
                        ps_u = ps1_p.tile([128, 512], mybir.dt.float32, name="ps1", tag="ps1")
                        for k in range(KB1):
                            nc.tensor.matmul(
                                out=ps_u[:, :nb],
                                lhsT=wgu_sb[e][k][
                                    :, _FL + i * 128 : _FL + (i + 1) * 128
                                ],
                                rhs=xs[k][:, off : off + nb],
                                start=(k == 0),
                                stop=(k == KB1 - 1),
                            )
                        ut = up_p.tile([128, 512], mybir.dt.bfloat16, name="up", tag="up")
                        nc.vector.tensor_copy(ut[:, :nb], ps_u[:, :nb])
                        ht = h_p.tile([128, 512], mybir.dt.bfloat16, name=f"h{i}", tag=f"h{i}")
                        nc.vector.tensor_mul(
                            out=ht[:, :nb], in0=ut[:, :nb], in1=st[:, :nb]
                        )
                        h_tiles.append(ht)
                    for hh in range(NH):
                        ps_y = ps2_p.tile([128, 512], mybir.dt.float32, name="ps2", tag="ps2")
                        for k in range(KB2):
                            nc.tensor.matmul(
                                out=ps_y[:, :nb],
                                lhsT=wd_sb[e][k][:, hh * 128 : (hh + 1) * 128],
                                rhs=h_tiles[k][:, :nb],
                                start=(k == 0),
                                stop=(k == KB2 - 1),
                            )
                        yt = y_p.tile([128, 512], mybir.dt.bfloat16, name="y", tag="y")
                        nc.vector.tensor_copy(yt[:, :nb], ps_y[:, :nb])
                        nc.sync.dma_start(
                            out=yT[hh * 128 : (hh + 1) * 128, s0 + off : s0 + off + nb],
                            in_=yt[:, :nb],
                        )
    nc.compile()
    return nc


def kernel(hidden_states, local_expert_indices, gate_up_proj, down_proj):
    from concourse.bass_utils import run_bass_kernel_spmd

    x = np.asarray(hidden_states, dtype=np.float32)
    idx = np.asarray(local_expert_indices).astype(np.int64)
    wgu_all = np.asarray(gate_up_proj, dtype=np.float32)
    wd_all = np.asarray(down_proj, dtype=np.float32)

    T, H = x.shape
    E, _, F2 = wgu_all.shape
    F = F2 // 2
    FL = F // _NC
    assert E == _NC and T == _T and H == _H and F == _F

    order = np.argsort(idx, kind="stable")
    counts = np.bincount(idx, minlength=E)

    key = tuple(int(c) for c in counts)
    if key not in _nc_cache:
        _nc_cache[key] = _build(key)
    nc = _nc_cache[key]

    xT = np.ascontiguousarray(x[order].T).astype(_BF16)
    wgu_bf = wgu_all.astype(_BF16)  # [E, H, 2F]
    wd_bf = wd_all.astype(_BF16)    # [E, F, H]

    in_maps = []
    for c in range(_NC):
        g = wgu_bf[:, :, c * FL : (c + 1) * FL]
        u = wgu_bf[:, :, F + c * FL : F + (c + 1) * FL]
        wgu_c = np.ascontiguousarray(
            np.concatenate([g, u], axis=2).reshape(E * H, 2 * FL)
        )
        wd_c = np.ascontiguousarray(wd_bf[:, c * FL : (c + 1) * FL, :]).reshape(
            E * FL, H
        )
        in_maps.append({"xT": xT, "wgu": wgu_c, "wd": wd_c})

    res = run_bass_kernel_spmd(nc, in_maps, core_ids=list(range(_NC)))
    global last_run
    last_run = res

    acc = np.zeros((H, T), np.float32)
    for c in range(_NC):
        acc += np.asarray(res.results[c]["yT"], dtype=np.float32)
    out = np.empty((T, H), np.float32)
    out[order] = acc.T
    return out


# revision 4
# speedup vs baseline: 1.1341x; 1.1341x over previous
"""Tensor-parallel (F-sharded) MoE MLP (Llama4 text experts) for 8 Trainium2 cores.

Strategy: shard the intermediate dimension F across the 8 cores. Core c holds,
for ALL 8 experts, gate columns [c*FL,(c+1)*FL) and up columns [F+c*FL, ...) of
gate_up_proj, plus rows [c*FL,(c+1)*FL) of down_proj (FL = F/8 = 256). Every
core processes ALL tokens (sorted by expert id): for each expert segment it
computes h = up * silu(gate) over its 256-wide F slice and a partial
y_c = h @ Wd_slice; the host sums the 8 partials. Segment widths are the global
expert counts - identical on every core - so one uniform SPMD program covers
all cores with zero token padding and perfect load balance.

DMA instruction count is minimized (the HWDGE descriptor generator costs
~630ns per dma_start, serialized): each x chunk, each expert weight block, and
each y chunk moves in ONE dma_start using a 3D access pattern that folds the
8 (or 2) 128-row k-chunks into the free dimension. x/weight loads issue on
the SP (sync) HWDGE queue, y stores on the Activation HWDGE queue.

Everything runs in the transposed layout (weights stationary, tokens stream):
h^T = up^T * silu(Wgu^T x^T), y^T = Wd^T h^T. bf16 in, fp32 PSUM, bf16 partial
out (host accumulates in fp32). The program is specialized to the expert-count
tuple (host knows counts before launch; compiled kernels are cached).
"""

import numpy as np
import ml_dtypes

_BF16 = ml_dtypes.bfloat16
_NC = 8           # cores
_T = 8192         # tokens
_H = 1024         # hidden
_F = 2048         # intermediate
_FL = _F // _NC   # 256 local F columns per core
_F2L = 2 * _FL    # 512 = [gate | up] local columns

_nc_cache: dict = {}
last_run = None  # BassKernelResults of the most recent kernel() call


def _chunks_of(n, cap=512):
    """Split n columns into near-equal chunks of at most cap."""
    if n <= 0:
        return []
    npc = -(-n // cap)
    base, rem = divmod(n, npc)
    sizes = [base + 1] * rem + [base] * (npc - rem)
    out, off = [], 0
    for s in sizes:
        out.append((off, s))
        off += s
    return out


def _build(counts):
    import concourse.bacc as bacc
    import concourse.mybir as mybir
    from concourse.tile import TileContext

    nc = bacc.Bacc()
    KB1 = _H // 128   # contraction chunks for x @ Wgu
    KB2 = _FL // 128  # contraction chunks for h @ Wd (= 2)
    NH = _H // 128    # output row tiles of y

    xT = nc.dram_tensor("xT", [_H, _T], mybir.dt.bfloat16, kind="ExternalInput")
    wgu = nc.dram_tensor(
        "wgu", [_NC * _H, _F2L], mybir.dt.bfloat16, kind="ExternalInput"
    )
    wd = nc.dram_tensor("wd", [_NC * _FL, _H], mybir.dt.bfloat16, kind="ExternalInput")
    yT = nc.dram_tensor("yT", [_H, _T], mybir.dt.bfloat16, kind="ExternalOutput")

    # 3D views folding 128-row k-chunks out of the row dim
    xT3 = xT[:, :].rearrange("(k p) t -> p k t", p=128)    # [128, KB1, T]
    yT3 = yT[:, :].rearrange("(hh p) t -> p hh t", p=128)  # [128, NH, T]

    # global chunk list: (expert, global t0, nb)
    chunks = []
    first_chunk_of_seg = {}  # expert -> chunk index
    t0 = 0
    seg_experts = []
    for e in range(_NC):
        n = int(counts[e])
        if n:
            first_chunk_of_seg[e] = len(chunks)
            seg_experts.append(e)
            for off, nb in _chunks_of(n):
                chunks.append((e, t0 + off, nb))
        t0 += n
    nch = len(chunks)

    with TileContext(nc) as tc:
        with (
            tc.tile_pool(name="wgu_p", bufs=1) as wgu_p,
            tc.tile_pool(name="wd_p", bufs=1) as wd_p,
            tc.tile_pool(name="x_p", bufs=4) as x_p,
            tc.tile_pool(name="silu_p", bufs=3) as silu_p,
            tc.tile_pool(name="up_p", bufs=3) as up_p,
            tc.tile_pool(name="h_p", bufs=2) as h_p,
            tc.tile_pool(name="y_p", bufs=3) as y_p,
            tc.tile_pool(name="ps1", bufs=4, space="PSUM") as ps1_p,
            tc.tile_pool(name="ps2", bufs=4, space="PSUM") as ps2_p,
        ):
            wgu_sb: dict = {}
            wd_sb: dict = {}
            x_sb: dict = {}

            def issue_w(e):
                if e in wgu_sb:
                    return
                gt = wgu_p.tile(
                    [128, KB1, _F2L], mybir.dt.bfloat16, name=f"wgu{e}", tag=f"wgu{e}"
                )
                nc.sync.dma_start(
                    out=gt,
                    in_=wgu[e * _H : (e + 1) * _H, :].rearrange(
                        "(k p) f -> p k f", p=128
                    ),
                )
                wgu_sb[e] = gt
                dt_ = wd_p.tile(
                    [128, KB2, _H], mybir.dt.bfloat16, name=f"wd{e}", tag=f"wd{e}"
                )
                nc.sync.dma_start(
                    out=dt_,
                    in_=wd[e * _FL : (e + 1) * _FL, :].rearrange(
                        "(k p) h -> p k h", p=128
                    ),
                )
                wd_sb[e] = dt_

            def issue_x(ci):
                e, t0, nb = chunks[ci]
                xt = x_p.tile([128, KB1, 512], mybir.dt.bfloat16, name=f"x{ci}", tag="x")
                nc.sync.dma_start(out=xt[:, :, :nb], in_=xT3[:, :, t0 : t0 + nb])
                x_sb[ci] = xt

            # prologue: first chunk's x + weights, then one chunk ahead
            issue_x(0)
            issue_w(chunks[0][0])
            if nch > 1:
                issue_x(1)
                issue_w(chunks[1][0])

            for ci in range(nch):
                e, t0, nb = chunks[ci]
                # prefetch: x two chunks ahead, weights for the expert of ci+2
                if ci + 2 < nch:
                    issue_x(ci + 2)
                    issue_w(chunks[ci + 2][0])
                xt = x_sb.pop(ci)
                h_tiles = []
                for i in range(2):  # two 128-row f-local tiles
                    ps_g = ps1_p.tile(
                        [128, 512], mybir.dt.float32, name="ps1", tag="ps1"
                    )
                    for k in range(KB1):
                        nc.tensor.matmul(
                            out=ps_g[:, :nb],
                            lhsT=wgu_sb[e][:, k, i * 128 : (i + 1) * 128],
                            rhs=xt[:, k, :nb],
                            start=(k == 0),
                            stop=(k == KB1 - 1),
                        )
                    st = silu_p.tile(
                        [128, 512], mybir.dt.bfloat16, name="silu", tag="silu"
                    )
                    nc.scalar.activation(
                        st[:, :nb], ps_g[:, :nb], mybir.ActivationFunctionType.Silu
                    )
                    ps_u = ps1_p.tile(
                        [128, 512], mybir.dt.float32, name="ps1", tag="ps1"
                    )
                    for k in range(KB1):
                        nc.tensor.matmul(
                            out=ps_u[:, :nb],
                            lhsT=wgu_sb[e][:, k, _FL + i * 128 : _FL + (i + 1) * 128],
                            rhs=xt[:, k, :nb],
                            start=(k == 0),
                            stop=(k == KB1 - 1),
                        )
                    ut = up_p.tile([128, 512], mybir.dt.bfloat16, name="up", tag="up")
                    nc.scalar.activation(
                        ut[:, :nb], ps_u[:, :nb], mybir.ActivationFunctionType.Copy
                    )
                    ht = h_p.tile(
                        [128, 512], mybir.dt.bfloat16, name=f"h{i}", tag=f"h{i}"
                    )
                    nc.vector.tensor_mul(out=ht[:, :nb], in0=ut[:, :nb], in1=st[:, :nb])
                    h_tiles.append(ht)
                yt = y_p.tile([128, NH, 512], mybir.dt.bfloat16, name="y", tag="y")
                for hh in range(NH):
                    ps_y = ps2_p.tile([128, 512], mybir.dt.float32, name="ps2", tag="ps2")
                    for k in range(KB2):
                        nc.tensor.matmul(
                            out=ps_y[:, :nb],
                            lhsT=wd_sb[e][:, k, hh * 128 : (hh + 1) * 128],
                            rhs=h_tiles[k][:, :nb],
                            start=(k == 0),
                            stop=(k == KB2 - 1),
                        )
                    nc.vector.tensor_copy(yt[:, hh, :nb], ps_y[:, :nb])
                nc.scalar.dma_start(
                    out=yT3[:, :, t0 : t0 + nb], in_=yt[:, :, :nb]
                )
    nc.compile()
    return nc


def kernel(hidden_states, local_expert_indices, gate_up_proj, down_proj):
    from concourse.bass_utils import run_bass_kernel_spmd

    x = np.asarray(hidden_states, dtype=np.float32)
    idx = np.asarray(local_expert_indices).astype(np.int64)
    wgu_all = np.asarray(gate_up_proj, dtype=np.float32)
    wd_all = np.asarray(down_proj, dtype=np.float32)

    T, H = x.shape
    E, _, F2 = wgu_all.shape
    F = F2 // 2
    FL = F // _NC
    assert E == _NC and T == _T and H == _H and F == _F

    order = np.argsort(idx, kind="stable")
    counts = np.bincount(idx, minlength=E)

    key = tuple(int(c) for c in counts)
    if key not in _nc_cache:
        _nc_cache[key] = _build(key)
    nc = _nc_cache[key]

    xT = np.ascontiguousarray(x[order].T).astype(_BF16)
    wgu_bf = wgu_all.astype(_BF16)  # [E, H, 2F]
    wd_bf = wd_all.astype(_BF16)    # [E, F, H]

    in_maps = []
    for c in range(_NC):
        g = wgu_bf[:, :, c * FL : (c + 1) * FL]
        u = wgu_bf[:, :, F + c * FL : F + (c + 1) * FL]
        wgu_c = np.ascontiguousarray(
            np.concatenate([g, u], axis=2).reshape(E * H, 2 * FL)
        )
        wd_c = np.ascontiguousarray(wd_bf[:, c * FL : (c + 1) * FL, :]).reshape(
            E * FL, H
        )
        in_maps.append({"xT": xT, "wgu": wgu_c, "wd": wd_c})

    res = run_bass_kernel_spmd(nc, in_maps, core_ids=list(range(_NC)))
    global last_run
    last_run = res

    acc = np.zeros((H, T), np.float32)
    for c in range(_NC):
        acc += np.asarray(res.results[c]["yT"], dtype=np.float32)
    out = np.empty((T, H), np.float32)
    out[order] = acc.T
    return out


# revision 5
# speedup vs baseline: 1.1419x; 1.0069x over previous
"""Tensor-parallel (F-sharded) MoE MLP (Llama4 text experts) for 8 Trainium2 cores.

Strategy: shard the intermediate dimension F across the 8 cores. Core c holds,
for ALL 8 experts, gate columns [c*FL,(c+1)*FL) and up columns [F+c*FL, ...) of
gate_up_proj, plus rows [c*FL,(c+1)*FL) of down_proj (FL = F/8 = 256). Every
core processes ALL tokens (sorted by expert id): for each expert segment it
computes h = up * silu(gate) over its 256-wide F slice and a partial
y_c = h @ Wd_slice; the host sums the 8 partials. Segment widths are the global
expert counts - identical on every core - so one uniform SPMD program covers
all cores with zero token padding and perfect load balance.

DMA design (the HWDGE descriptor generator costs ~630ns per dma_start,
serialized, and per-queue bandwidth depends on line size):
  - x loads: one SWDGE dma_start per token chunk on gpsimd (own queue),
    3D AP folds the 8 128-row k-chunks into the free dim.
  - weights: one dma_start per expert tensor on the SP (sync) HWDGE queue.
  - y stores: one dma_start per chunk on the Activation HWDGE queue, writing
    a chunk-packed contiguous DRAM layout yP (lines of 8*nb*2 bytes); the
    host unpacks. First/last chunks are 128 tokens to shorten head/tail.

Everything runs in the transposed layout (weights stationary, tokens stream):
h^T = silu(Wgu_g^T x^T) * (Wgu_u^T x^T), y^T = Wd^T h^T. bf16 in, fp32 PSUM,
bf16 partial out (host accumulates in fp32). The program is specialized to the
expert-count tuple (host knows counts before launch; compiled kernels cached).
"""

import numpy as np
import ml_dtypes

_BF16 = ml_dtypes.bfloat16
_NC = 8           # cores
_T = 8192         # tokens
_H = 1024         # hidden
_F = 2048         # intermediate
_FL = _F // _NC   # 256 local F columns per core
_F2L = 2 * _FL    # 512 = [gate | up] local columns
_NH = _H // 128   # 8 output row tiles

_nc_cache: dict = {}
last_run = None  # BassKernelResults of the most recent kernel() call


def _chunks_of(n, cap=512):
    """Split n columns into near-equal chunks of at most cap."""
    if n <= 0:
        return []
    npc = -(-n // cap)
    base, rem = divmod(n, npc)
    sizes = [base + 1] * rem + [base] * (npc - rem)
    out, off = [], 0
    for s in sizes:
        out.append((off, s))
        off += s
    return out


def _chunk_list(counts):
    """Global chunk list [(expert, t0, nb)]; small first/last chunks."""
    chunks = []
    t0 = 0
    for e in range(_NC):
        n = int(counts[e])
        for off, nb in _chunks_of(n):
            chunks.append((e, t0 + off, nb))
        t0 += n
    # split a 128-token prefix off the first chunk and suffix off the last
    if chunks and chunks[0][2] > 256:
        e, t0, nb = chunks[0]
        chunks[0:1] = [(e, t0, 128), (e, t0 + 128, nb - 128)]
    if chunks and chunks[-1][2] > 256:
        e, t0, nb = chunks[-1]
        chunks[-1:] = [(e, t0, nb - 128), (e, t0 + nb - 128, 128)]
    return chunks


def _build(counts):
    import concourse.bacc as bacc
    import concourse.mybir as mybir
    from concourse.tile import TileContext

    nc = bacc.Bacc()
    KB1 = _H // 128   # contraction chunks for x @ Wgu
    KB2 = _FL // 128  # contraction chunks for h @ Wd (= 2)

    xT = nc.dram_tensor("xT", [_H, _T], mybir.dt.bfloat16, kind="ExternalInput")
    wgu = nc.dram_tensor(
        "wgu", [_NC * _H, _F2L], mybir.dt.bfloat16, kind="ExternalInput"
    )
    wd = nc.dram_tensor("wd", [_NC * _FL, _H], mybir.dt.bfloat16, kind="ExternalInput")
    yP = nc.dram_tensor("yP", [128, _NH * _T], mybir.dt.bfloat16, kind="ExternalOutput")

    xT3 = xT[:, :].rearrange("(k p) t -> p k t", p=128)  # [128, KB1, T]

    chunks = _chunk_list(counts)
    nch = len(chunks)

    with TileContext(nc) as tc:
        with (
            tc.tile_pool(name="wgu_p", bufs=1) as wgu_p,
            tc.tile_pool(name="wd_p", bufs=1) as wd_p,
            tc.tile_pool(name="x_p", bufs=4) as x_p,
            tc.tile_pool(name="silu_p", bufs=3) as silu_p,
            tc.tile_pool(name="h_p", bufs=2) as h_p,
            tc.tile_pool(name="y_p", bufs=3) as y_p,
            tc.tile_pool(name="ps1", bufs=4, space="PSUM") as ps1_p,
            tc.tile_pool(name="ps2", bufs=4, space="PSUM") as ps2_p,
        ):
            wgu_sb: dict = {}
            wd_sb: dict = {}
            x_sb: dict = {}

            def issue_w(e):
                if e in wgu_sb:
                    return
                gt = wgu_p.tile(
                    [128, KB1, _F2L], mybir.dt.bfloat16, name=f"wgu{e}", tag=f"wgu{e}"
                )
                nc.sync.dma_start(
                    out=gt,
                    in_=wgu[e * _H : (e + 1) * _H, :].rearrange(
                        "(k p) f -> p k f", p=128
                    ),
                )
                wgu_sb[e] = gt
                dt_ = wd_p.tile(
                    [128, KB2, _H], mybir.dt.bfloat16, name=f"wd{e}", tag=f"wd{e}"
                )
                nc.sync.dma_start(
                    out=dt_,
                    in_=wd[e * _FL : (e + 1) * _FL, :].rearrange(
                        "(k p) h -> p k h", p=128
                    ),
                )
                wd_sb[e] = dt_

            def issue_x(ci):
                e, t0, nb = chunks[ci]
                xt = x_p.tile(
                    [128, KB1, 512], mybir.dt.bfloat16, name=f"x{ci}", tag="x"
                )
                nc.gpsimd.dma_start(out=xt[:, :, :nb], in_=xT3[:, :, t0 : t0 + nb])
                x_sb[ci] = xt

            # prologue: first chunk's x + weights, then one chunk ahead
            issue_x(0)
            issue_w(chunks[0][0])
            if nch > 1:
                issue_x(1)
                issue_w(chunks[1][0])

            for ci in range(nch):
                e, t0, nb = chunks[ci]
                if ci + 2 < nch:
                    issue_x(ci + 2)
                    issue_w(chunks[ci + 2][0])
                xt = x_sb.pop(ci)
                h_tiles = []
                for i in range(2):  # two 128-row f-local tiles
                    ps_g = ps1_p.tile(
                        [128, 512], mybir.dt.float32, name="ps1", tag="ps1"
                    )
                    for k in range(KB1):
                        nc.tensor.matmul(
                            out=ps_g[:, :nb],
                            lhsT=wgu_sb[e][:, k, i * 128 : (i + 1) * 128],
                            rhs=xt[:, k, :nb],
                            start=(k == 0),
                            stop=(k == KB1 - 1),
                        )
                    st = silu_p.tile(
                        [128, 512], mybir.dt.bfloat16, name="silu", tag="silu"
                    )
                    nc.scalar.activation(
                        st[:, :nb], ps_g[:, :nb], mybir.ActivationFunctionType.Silu
                    )
                    ps_u = ps1_p.tile(
                        [128, 512], mybir.dt.float32, name="ps1", tag="ps1"
                    )
                    for k in range(KB1):
                        nc.tensor.matmul(
                            out=ps_u[:, :nb],
                            lhsT=wgu_sb[e][:, k, _FL + i * 128 : _FL + (i + 1) * 128],
                            rhs=xt[:, k, :nb],
                            start=(k == 0),
                            stop=(k == KB1 - 1),
                        )
                    ht = h_p.tile(
                        [128, 512], mybir.dt.bfloat16, name=f"h{i}", tag=f"h{i}"
                    )
                    nc.vector.tensor_mul(
                        out=ht[:, :nb], in0=ps_u[:, :nb], in1=st[:, :nb]
                    )
                    h_tiles.append(ht)
                yt = y_p.tile([128, _NH * 512], mybir.dt.bfloat16, name="y", tag="y")
                for g in range(2):  # hh groups of 4; k=0 first to hide ht1 latency
                    pss = []
                    for j in range(4):
                        ps_y = ps2_p.tile(
                            [128, 512], mybir.dt.float32, name="ps2", tag="ps2"
                        )
                        hh = g * 4 + j
                        nc.tensor.matmul(
                            out=ps_y[:, :nb],
                            lhsT=wd_sb[e][:, 0, hh * 128 : (hh + 1) * 128],
                            rhs=h_tiles[0][:, :nb],
                            start=True,
                            stop=False,
                        )
                        pss.append(ps_y)
                    for j in range(4):
                        hh = g * 4 + j
                        nc.tensor.matmul(
                            out=pss[j][:, :nb],
                            lhsT=wd_sb[e][:, 1, hh * 128 : (hh + 1) * 128],
                            rhs=h_tiles[1][:, :nb],
                            start=False,
                            stop=True,
                        )
                        nc.vector.tensor_copy(
                            yt[:, hh * nb : (hh + 1) * nb], pss[j][:, :nb]
                        )
                nc.scalar.dma_start(
                    out=yP[:, _NH * t0 : _NH * (t0 + nb)], in_=yt[:, : _NH * nb]
                )
    nc.compile()
    return nc


def kernel(hidden_states, local_expert_indices, gate_up_proj, down_proj):
    from concourse.bass_utils import run_bass_kernel_spmd

    x = np.asarray(hidden_states, dtype=np.float32)
    idx = np.asarray(local_expert_indices).astype(np.int64)
    wgu_all = np.asarray(gate_up_proj, dtype=np.float32)
    wd_all = np.asarray(down_proj, dtype=np.float32)

    T, H = x.shape
    E, _, F2 = wgu_all.shape
    F = F2 // 2
    FL = F // _NC
    assert E == _NC and T == _T and H == _H and F == _F

    order = np.argsort(idx, kind="stable")
    counts = np.bincount(idx, minlength=E)

    key = tuple(int(c) for c in counts)
    if key not in _nc_cache:
        _nc_cache[key] = _build(key)
    nc = _nc_cache[key]

    xT = np.ascontiguousarray(x[order].T).astype(_BF16)
    wgu_bf = wgu_all.astype(_BF16)  # [E, H, 2F]
    wd_bf = wd_all.astype(_BF16)    # [E, F, H]

    in_maps = []
    for c in range(_NC):
        g = wgu_bf[:, :, c * FL : (c + 1) * FL]
        u = wgu_bf[:, :, F + c * FL : F + (c + 1) * FL]
        wgu_c = np.ascontiguousarray(
            np.concatenate([g, u], axis=2).reshape(E * H, 2 * FL)
        )
        wd_c = np.ascontiguousarray(wd_bf[:, c * FL : (c + 1) * FL, :]).reshape(
            E * FL, H
        )
        in_maps.append({"xT": xT, "wgu": wgu_c, "wd": wd_c})

    res = run_bass_kernel_spmd(nc, in_maps, core_ids=list(range(_NC)))
    global last_run
    last_run = res

    # unpack chunk-packed yP [128, NH*T] and accumulate partials in fp32
    chunks = _chunk_list(counts)
    acc = np.zeros((H, T), np.float32)
    for c in range(_NC):
        yp = np.asarray(res.results[c]["yP"], dtype=np.float32)
        for e, t0, nb in chunks:
            blk = yp[:, _NH * t0 : _NH * (t0 + nb)].reshape(128, _NH, nb)
            acc[:, t0 : t0 + nb] += blk.transpose(1, 0, 2).reshape(H, nb)
    out = np.empty((T, H), np.float32)
    out[order] = acc.T
    return out


# revision 6
# speedup vs baseline: 1.1810x; 1.0342x over previous
"""Tensor-parallel (F-sharded) MoE MLP (Llama4 text experts) for 8 Trainium2 cores.

Strategy: shard the intermediate dimension F across the 8 cores. Core c holds,
for ALL 8 experts, gate columns [c*FL,(c+1)*FL) and up columns [F+c*FL, ...) of
gate_up_proj, plus rows [c*FL,(c+1)*FL) of down_proj (FL = F/8 = 256). Every
core processes ALL tokens (sorted by expert id): for each expert segment it
computes h = up * silu(gate) over its 256-wide F slice and a partial
y_c = h @ Wd_slice; the host sums the 8 partials. Segment widths are the global
expert counts - identical on every core - so one uniform SPMD program covers
all cores with zero token padding and perfect load balance.

DMA design: every transfer is contiguous-to-contiguous with one dma_start
(the HWDGE descriptor generator costs ~630ns per instruction and queues want
>=4KB lines). The host pre-packs DRAM layouts to match the SBUF tiles exactly:
  xP  [128, KB1*T]   chunk-packed: chunk (t0,nb) at cols KB1*t0, layout [k][t]
  wgu [E*128, KB1*F2L] per-expert [p][k][f] flat
  wd  [E*128, KB2*H]   per-expert [p][k][h] flat
  yP  [128, NH*T]    chunk-packed partial output, layout [hh][t] per chunk
x loads ride the gpsimd SWDGE queue (first two on sync for a fast start),
weights and y stores ride the SP HWDGE queue, silu plus 3 of 8 PSUM-evict
casts ride the Activation engine, the rest of the casts and the h-mul ride
the DVE. L2 issues its k=0 matmuls for 4 output tiles first so the PE has
work while the second h tile finishes.

bf16 in, fp32 PSUM, bf16 partial out (host accumulates in fp32). The program
is specialized to the expert-count tuple (cached per counts).
"""

import numpy as np
import ml_dtypes

_BF16 = ml_dtypes.bfloat16
_NC = 8           # cores
_T = 8192         # tokens
_H = 1024         # hidden
_F = 2048         # intermediate
_FL = _F // _NC   # 256 local F columns per core
_F2L = 2 * _FL    # 512 = [gate | up] local columns
_NH = _H // 128   # 8 output row tiles
_KB1 = _H // 128  # 8 contraction chunks for x @ Wgu
_KB2 = _FL // 128  # 2 contraction chunks for h @ Wd

_ACT_HH = (1, 4, 7)  # y-cast tiles evicted by the Activation engine

_nc_cache: dict = {}
last_run = None  # BassKernelResults of the most recent kernel() call


def _chunks_of(n, cap=512):
    """Split n columns into near-equal chunks of at most cap."""
    if n <= 0:
        return []
    npc = -(-n // cap)
    base, rem = divmod(n, npc)
    sizes = [base + 1] * rem + [base] * (npc - rem)
    out, off = [], 0
    for s in sizes:
        out.append((off, s))
        off += s
    return out


def _chunk_list(counts):
    """Global chunk list [(expert, t0, nb)]; small first/last chunks."""
    chunks = []
    t0 = 0
    for e in range(_NC):
        n = int(counts[e])
        for off, nb in _chunks_of(n):
            chunks.append((e, t0 + off, nb))
        t0 += n
    if chunks and chunks[0][2] > 256:
        e, t0, nb = chunks[0]
        chunks[0:1] = [(e, t0, 128), (e, t0 + 128, nb - 128)]
    if chunks and chunks[-1][2] > 256:
        e, t0, nb = chunks[-1]
        chunks[-1:] = [(e, t0, nb - 128), (e, t0 + nb - 128, 128)]
    return chunks


def _build(counts):
    import concourse.bacc as bacc
    import concourse.mybir as mybir
    from concourse.tile import TileContext

    nc = bacc.Bacc()

    xP = nc.dram_tensor("xP", [128, _KB1 * _T], mybir.dt.bfloat16, kind="ExternalInput")
    wgu = nc.dram_tensor(
        "wgu", [_NC * 128, _KB1 * _F2L], mybir.dt.bfloat16, kind="ExternalInput"
    )
    wd = nc.dram_tensor(
        "wd", [_NC * 128, _KB2 * _H], mybir.dt.bfloat16, kind="ExternalInput"
    )
    yP = nc.dram_tensor("yP", [128, _NH * _T], mybir.dt.bfloat16, kind="ExternalOutput")

    chunks = _chunk_list(counts)
    nch = len(chunks)

    with TileContext(nc) as tc:
        with (
            tc.tile_pool(name="wgu_p", bufs=1) as wgu_p,
            tc.tile_pool(name="wd_p", bufs=1) as wd_p,
            tc.tile_pool(name="x_p", bufs=4) as x_p,
            tc.tile_pool(name="silu_p", bufs=4) as silu_p,
            tc.tile_pool(name="h_p", bufs=3) as h_p,
            tc.tile_pool(name="y_p", bufs=3) as y_p,
            tc.tile_pool(name="ps1", bufs=4, space="PSUM") as ps1_p,
            tc.tile_pool(name="ps2", bufs=4, space="PSUM") as ps2_p,
        ):
            wgu_sb: dict = {}
            wd_sb: dict = {}
            x_sb: dict = {}

            def issue_w(e):
                if e in wgu_sb:
                    return
                gt = wgu_p.tile(
                    [128, _KB1 * _F2L],
                    mybir.dt.bfloat16,
                    name=f"wgu{e}",
                    tag=f"wgu{e}",
                )
                nc.sync.dma_start(out=gt, in_=wgu[e * 128 : (e + 1) * 128, :])
                wgu_sb[e] = gt
                dt_ = wd_p.tile(
                    [128, _KB2 * _H], mybir.dt.bfloat16, name=f"wd{e}", tag=f"wd{e}"
                )
                nc.sync.dma_start(out=dt_, in_=wd[e * 128 : (e + 1) * 128, :])
                wd_sb[e] = dt_

            def issue_x(ci, eng):
                e, t0, nb = chunks[ci]
                xt = x_p.tile(
                    [128, _KB1 * 512], mybir.dt.bfloat16, name=f"x{ci}", tag="x"
                )
                eng.dma_start(
                    out=xt[:, : _KB1 * nb],
                    in_=xP[:, _KB1 * t0 : _KB1 * (t0 + nb)],
                )
                x_sb[ci] = xt

            # prologue: first two chunks' x on the fast HWDGE queue + weights
            issue_x(0, nc.sync)
            issue_w(chunks[0][0])
            if nch > 1:
                issue_x(1, nc.sync)
                issue_w(chunks[1][0])
            if nch > 2:
                issue_x(2, nc.gpsimd)
                issue_w(chunks[2][0])

            for ci in range(nch):
                e, t0, nb = chunks[ci]
                if ci + 3 < nch:
                    issue_x(ci + 3, nc.gpsimd)
                    issue_w(chunks[ci + 3][0])
                xt = x_sb.pop(ci)
                h_tiles = []
                for i in range(2):  # two 128-row f-local tiles
                    ps_g = ps1_p.tile(
                        [128, 512], mybir.dt.float32, name="ps1", tag="ps1"
                    )
                    for k in range(_KB1):
                        nc.tensor.matmul(
                            out=ps_g[:, :nb],
                            lhsT=wgu_sb[e][
                                :, k * _F2L + i * 128 : k * _F2L + (i + 1) * 128
                            ],
                            rhs=xt[:, k * nb : (k + 1) * nb],
                            start=(k == 0),
                            stop=(k == _KB1 - 1),
                        )
                    st = silu_p.tile(
                        [128, 512], mybir.dt.bfloat16, name="silu", tag="silu"
                    )
                    nc.scalar.activation(
                        st[:, :nb], ps_g[:, :nb], mybir.ActivationFunctionType.Silu
                    )
                    ps_u = ps1_p.tile(
                        [128, 512], mybir.dt.float32, name="ps1", tag="ps1"
                    )
                    for k in range(_KB1):
                        nc.tensor.matmul(
                            out=ps_u[:, :nb],
                            lhsT=wgu_sb[e][
                                :,
                                k * _F2L + _FL + i * 128 : k * _F2L
                                + _FL
                                + (i + 1) * 128,
                            ],
                            rhs=xt[:, k * nb : (k + 1) * nb],
                            start=(k == 0),
                            stop=(k == _KB1 - 1),
                        )
                    ht = h_p.tile(
                        [128, 512], mybir.dt.bfloat16, name=f"h{i}", tag=f"h{i}"
                    )
                    nc.vector.tensor_mul(
                        out=ht[:, :nb], in0=ps_u[:, :nb], in1=st[:, :nb]
                    )
                    h_tiles.append(ht)
                yt = y_p.tile([128, _NH * 512], mybir.dt.bfloat16, name="y", tag="y")
                for g in range(2):  # hh groups of 4; k=0 first to hide ht1 latency
                    pss = []
                    for j in range(4):
                        ps_y = ps2_p.tile(
                            [128, 512], mybir.dt.float32, name="ps2", tag="ps2"
                        )
                        hh = g * 4 + j
                        nc.tensor.matmul(
                            out=ps_y[:, :nb],
                            lhsT=wd_sb[e][:, hh * 128 : (hh + 1) * 128],
                            rhs=h_tiles[0][:, :nb],
                            start=True,
                            stop=False,
                        )
                        pss.append(ps_y)
                    for j in range(4):
                        hh = g * 4 + j
                        nc.tensor.matmul(
                            out=pss[j][:, :nb],
                            lhsT=wd_sb[e][:, _H + hh * 128 : _H + (hh + 1) * 128],
                            rhs=h_tiles[1][:, :nb],
                            start=False,
                            stop=True,
                        )
                        dst = yt[:, hh * nb : (hh + 1) * nb]
                        if hh in _ACT_HH:
                            nc.scalar.activation(
                                dst, pss[j][:, :nb], mybir.ActivationFunctionType.Copy
                            )
                        else:
                            nc.vector.tensor_copy(dst, pss[j][:, :nb])
                nc.sync.dma_start(
                    out=yP[:, _NH * t0 : _NH * (t0 + nb)], in_=yt[:, : _NH * nb]
                )
    nc.compile()
    return nc


def kernel(hidden_states, local_expert_indices, gate_up_proj, down_proj):
    from concourse.bass_utils import run_bass_kernel_spmd

    x = np.asarray(hidden_states, dtype=np.float32)
    idx = np.asarray(local_expert_indices).astype(np.int64)
    wgu_all = np.asarray(gate_up_proj, dtype=np.float32)
    wd_all = np.asarray(down_proj, dtype=np.float32)

    T, H = x.shape
    E, _, F2 = wgu_all.shape
    F = F2 // 2
    FL = F // _NC
    assert E == _NC and T == _T and H == _H and F == _F

    order = np.argsort(idx, kind="stable")
    counts = np.bincount(idx, minlength=E)

    key = tuple(int(c) for c in counts)
    if key not in _nc_cache:
        _nc_cache[key] = _build(key)
    nc = _nc_cache[key]

    chunks = _chunk_list(counts)

    # x packed: [128, KB1*T], chunk (t0, nb) occupies cols KB1*t0 .. KB1*(t0+nb)
    # laid out [k][t] (so the SBUF tile slice k*nb:(k+1)*nb is the k-th chunk)
    xs = np.asarray(x[order].T, dtype=_BF16)         # [H, T]
    xv = xs.reshape(_KB1, 128, T).transpose(1, 0, 2)  # [128, k, T]
    xP = np.empty((128, _KB1 * T), dtype=_BF16)
    for e_, t0, nb in chunks:
        xP[:, _KB1 * t0 : _KB1 * (t0 + nb)] = xv[:, :, t0 : t0 + nb].reshape(
            128, _KB1 * nb
        )

    wgu_bf = wgu_all.astype(_BF16)  # [E, H, 2F]
    wd_bf = wd_all.astype(_BF16)    # [E, F, H]

    in_maps = []
    for c in range(_NC):
        g = wgu_bf[:, :, c * FL : (c + 1) * FL]
        u = wgu_bf[:, :, F + c * FL : F + (c + 1) * FL]
        wgu_c = np.concatenate([g, u], axis=2)        # [E, H, F2L]
        # -> [E, p, k, f] flat as [E*128, KB1*F2L]
        wgu_pk = wgu_c.reshape(E, _KB1, 128, _F2L).transpose(0, 2, 1, 3)
        wgu_p = np.ascontiguousarray(wgu_pk).reshape(E * 128, _KB1 * _F2L)
        wd_c = wd_bf[:, c * FL : (c + 1) * FL, :]     # [E, FL, H]
        wd_pk = wd_c.reshape(E, _KB2, 128, H).transpose(0, 2, 1, 3)
        wd_p = np.ascontiguousarray(wd_pk).reshape(E * 128, _KB2 * H)
        in_maps.append({"xP": xP, "wgu": wgu_p, "wd": wd_p})

    res = run_bass_kernel_spmd(nc, in_maps, core_ids=list(range(_NC)))
    global last_run
    last_run = res

    # unpack chunk-packed yP [128, NH*T] and accumulate partials in fp32
    acc = np.zeros((H, T), np.float32)
    for c in range(_NC):
        yp = np.asarray(res.results[c]["yP"], dtype=np.float32)
        for e_, t0, nb in chunks:
            blk = yp[:, _NH * t0 : _NH * (t0 + nb)].reshape(128, _NH, nb)
            acc[:, t0 : t0 + nb] += blk.transpose(1, 0, 2).reshape(H, nb)
    out = np.empty((T, H), np.float32)
    out[order] = acc.T
    return out


# revision 10
# speedup vs baseline: 1.1852x; 1.0035x over previous
"""Tensor-parallel (F-sharded) MoE MLP (Llama4 text experts) for 8 Trainium2 cores.

Strategy: shard the intermediate dimension F across the 8 cores. Core c holds,
for ALL 8 experts, gate columns [c*FL,(c+1)*FL) and up columns [F+c*FL, ...) of
gate_up_proj, plus rows [c*FL,(c+1)*FL) of down_proj (FL = F/8 = 256). Every
core processes ALL tokens (sorted by expert id): for each expert segment it
computes h = up * silu(gate) over its 256-wide F slice and a partial
y_c = h @ Wd_slice; the host sums the 8 partials. Segment widths are the global
expert counts - identical on every core - so one uniform SPMD program covers
all cores with zero token padding and perfect load balance.

DMA design: every transfer is contiguous-to-contiguous with one dma_start
(the HWDGE descriptor generator costs ~630ns per instruction and queues want
>=4KB lines). The host pre-packs DRAM layouts to match the SBUF tiles exactly:
  xP  [128, KB1*T]   chunk-packed: chunk (t0,nb) at cols KB1*t0, layout [k][t]
  wgu [E*128, KB1*F2L] per-expert [p][k][f] flat
  wd  [E*128, KB2*H]   per-expert [p][k][h] flat
  yP  [128, NH*T]    chunk-packed partial output, layout [hh][t] per chunk
x loads ride the gpsimd SWDGE queue (first two on sync for a fast start),
weights and y stores ride the SP HWDGE queue, silu plus 3 of 8 PSUM-evict
casts ride the Activation engine, the rest of the casts and the h-mul ride
the DVE. L2 issues its k=0 matmuls for 4 output tiles first so the PE has
work while the second h tile finishes.

bf16 in, fp32 PSUM, bf16 partial out (host accumulates in fp32). The program
is specialized to the expert-count tuple (cached per counts).
"""

import numpy as np
import ml_dtypes

_BF16 = ml_dtypes.bfloat16
_NC = 8           # cores
_T = 8192         # tokens
_H = 1024         # hidden
_F = 2048         # intermediate
_FL = _F // _NC   # 256 local F columns per core
_F2L = 2 * _FL    # 512 = [gate | up] local columns
_NH = _H // 128   # 8 output row tiles
_KB1 = _H // 128  # 8 contraction chunks for x @ Wgu
_KB2 = _FL // 128  # 2 contraction chunks for h @ Wd

_ACT_HH = (1, 4, 7)  # y-cast tiles evicted by the Activation engine

_nc_cache: dict = {}
last_run = None  # BassKernelResults of the most recent kernel() call


def _chunks_of(n, cap=512):
    """Split n columns into near-equal chunks of at most cap."""
    if n <= 0:
        return []
    npc = -(-n // cap)
    base, rem = divmod(n, npc)
    sizes = [base + 1] * rem + [base] * (npc - rem)
    out, off = [], 0
    for s in sizes:
        out.append((off, s))
        off += s
    return out


def _chunk_list(counts):
    """Global chunk list [(expert, t0, nb)]; ramp-up head, small tail."""
    chunks = []
    t0 = 0
    for e in range(_NC):
        n = int(counts[e])
        for off, nb in _chunks_of(n):
            chunks.append((e, t0 + off, nb))
        t0 += n
    # ramp the first chunk up in small steps so compute starts early
    if chunks and chunks[0][2] > 384:
        e, t0, nb = chunks[0]
        head = [128, 128, 256]
        reps, off = [], 0
        for hsz in head:
            if nb - off <= hsz + 128:
                break
            reps.append((e, t0 + off, hsz))
            off += hsz
        reps.append((e, t0 + off, nb - off))
        chunks[0:1] = reps
    if chunks and chunks[-1][2] > 256:
        e, t0, nb = chunks[-1]
        chunks[-1:] = [(e, t0, nb - 128), (e, t0 + nb - 128, 128)]
    return chunks


def _build(counts):
    import concourse.bacc as bacc
    import concourse.mybir as mybir
    from concourse.tile import TileContext

    nc = bacc.Bacc()

    xP = nc.dram_tensor("xP", [128, _KB1 * _T], mybir.dt.bfloat16, kind="ExternalInput")
    wgu = nc.dram_tensor(
        "wgu", [_NC * 128, _KB1 * _F2L], mybir.dt.bfloat16, kind="ExternalInput"
    )
    wd = nc.dram_tensor(
        "wd", [_NC * 128, _KB2 * _H], mybir.dt.bfloat16, kind="ExternalInput"
    )
    yP = nc.dram_tensor("yP", [128, _NH * _T], mybir.dt.bfloat16, kind="ExternalOutput")

    chunks = _chunk_list(counts)
    nch = len(chunks)

    with TileContext(nc) as tc:
        with (
            tc.tile_pool(name="wgu_p", bufs=1) as wgu_p,
            tc.tile_pool(name="wd_p", bufs=1) as wd_p,
            tc.tile_pool(name="x_p", bufs=4) as x_p,
            tc.tile_pool(name="silu_p", bufs=4) as silu_p,
            tc.tile_pool(name="h_p", bufs=3) as h_p,
            tc.tile_pool(name="y_p", bufs=3) as y_p,
            tc.tile_pool(name="ps1", bufs=4, space="PSUM") as ps1_p,
            tc.tile_pool(name="ps2", bufs=4, space="PSUM") as ps2_p,
        ):
            wgu_sb: dict = {}
            wd_sb: dict = {}
            x_sb: dict = {}

            def issue_wgu(e):
                if e in wgu_sb:
                    return
                gt = wgu_p.tile(
                    [128, _KB1 * _F2L],
                    mybir.dt.bfloat16,
                    name=f"wgu{e}",
                    tag=f"wgu{e}",
                )
                nc.sync.dma_start(out=gt, in_=wgu[e * 128 : (e + 1) * 128, :])
                wgu_sb[e] = gt

            def issue_wd(e):
                if e in wd_sb:
                    return
                dt_ = wd_p.tile(
                    [128, _KB2 * _H], mybir.dt.bfloat16, name=f"wd{e}", tag=f"wd{e}"
                )
                nc.sync.dma_start(out=dt_, in_=wd[e * 128 : (e + 1) * 128, :])
                wd_sb[e] = dt_

            def issue_w(e):
                issue_wgu(e)
                issue_wd(e)

            def issue_x(ci, eng):
                e, t0, nb = chunks[ci]
                xt = x_p.tile(
                    [128, _KB1 * 512], mybir.dt.bfloat16, name=f"x{ci}", tag="x"
                )
                eng.dma_start(
                    out=xt[:, : _KB1 * nb],
                    in_=xP[:, _KB1 * t0 : _KB1 * (t0 + nb)],
                )
                x_sb[ci] = xt

            # prologue: first expert's wgu, then the small head chunks' x on
            # the fast HWDGE queue; wd0 trails (needed only at L2 of chunk 0)
            issue_wgu(chunks[0][0])
            for ci0 in range(min(3, nch)):
                issue_x(ci0, nc.sync)
            issue_wd(chunks[0][0])
            for ci0 in range(1, min(3, nch)):
                issue_w(chunks[ci0][0])

            for ci in range(nch):
                e, t0, nb = chunks[ci]
                if ci + 3 < nch:
                    issue_x(ci + 3, nc.gpsimd)
                    issue_w(chunks[ci + 3][0])
                xt = x_sb.pop(ci)
                h_tiles = []
                for i in range(2):  # two 128-row f-local tiles
                    ps_g = ps1_p.tile(
                        [128, 512], mybir.dt.float32, name="ps1", tag="ps1"
                    )
                    for k in range(_KB1):
                        nc.tensor.matmul(
                            out=ps_g[:, :nb],
                            lhsT=wgu_sb[e][
                                :, k * _F2L + i * 128 : k * _F2L + (i + 1) * 128
                            ],
                            rhs=xt[:, k * nb : (k + 1) * nb],
                            start=(k == 0),
                            stop=(k == _KB1 - 1),
                        )
                    st = silu_p.tile(
                        [128, 512], mybir.dt.bfloat16, name="silu", tag="silu"
                    )
                    nc.scalar.activation(
                        st[:, :nb], ps_g[:, :nb], mybir.ActivationFunctionType.Silu
                    )
                    ps_u = ps1_p.tile(
                        [128, 512], mybir.dt.float32, name="ps1", tag="ps1"
                    )
                    for k in range(_KB1):
                        nc.tensor.matmul(
                            out=ps_u[:, :nb],
                            lhsT=wgu_sb[e][
                                :,
                                k * _F2L + _FL + i * 128 : k * _F2L
                                + _FL
                                + (i + 1) * 128,
                            ],
                            rhs=xt[:, k * nb : (k + 1) * nb],
                            start=(k == 0),
                            stop=(k == _KB1 - 1),
                        )
                    ht = h_p.tile(
                        [128, 512], mybir.dt.bfloat16, name=f"h{i}", tag=f"h{i}"
                    )
                    nc.vector.tensor_mul(
                        out=ht[:, :nb], in0=ps_u[:, :nb], in1=st[:, :nb]
                    )
                    h_tiles.append(ht)
                yt = y_p.tile([128, _NH * 512], mybir.dt.bfloat16, name="y", tag="y")
                for g in range(2):  # hh groups of 4; k=0 first to hide ht1 latency
                    pss = []
                    for j in range(4):
                        ps_y = ps2_p.tile(
                            [128, 512], mybir.dt.float32, name="ps2", tag="ps2"
                        )
                        hh = g * 4 + j
                        nc.tensor.matmul(
                            out=ps_y[:, :nb],
                            lhsT=wd_sb[e][:, hh * 128 : (hh + 1) * 128],
                            rhs=h_tiles[0][:, :nb],
                            start=True,
                            stop=False,
                        )
                        pss.append(ps_y)
                    for j in range(4):
                        hh = g * 4 + j
                        nc.tensor.matmul(
                            out=pss[j][:, :nb],
                            lhsT=wd_sb[e][:, _H + hh * 128 : _H + (hh + 1) * 128],
                            rhs=h_tiles[1][:, :nb],
                            start=False,
                            stop=True,
                        )
                        dst = yt[:, hh * nb : (hh + 1) * nb]
                        if hh in _ACT_HH:
                            nc.scalar.activation(
                                dst, pss[j][:, :nb], mybir.ActivationFunctionType.Copy
                            )
                        else:
                            nc.vector.tensor_copy(dst, pss[j][:, :nb])
                nc.sync.dma_start(
                    out=yP[:, _NH * t0 : _NH * (t0 + nb)], in_=yt[:, : _NH * nb]
                )
    nc.compile()
    return nc


def kernel(hidden_states, local_expert_indices, gate_up_proj, down_proj):
    from concourse.bass_utils import run_bass_kernel_spmd

    x = np.asarray(hidden_states, dtype=np.float32)
    idx = np.asarray(local_expert_indices).astype(np.int64)
    wgu_all = np.asarray(gate_up_proj, dtype=np.float32)
    wd_all = np.asarray(down_proj, dtype=np.float32)

    T, H = x.shape
    E, _, F2 = wgu_all.shape
    F = F2 // 2
    FL = F // _NC
    assert E == _NC and T == _T and H == _H and F == _F

    order = np.argsort(idx, kind="stable")
    counts = np.bincount(idx, minlength=E)

    key = tuple(int(c) for c in counts)
    if key not in _nc_cache:
        _nc_cache[key] = _build(key)
    nc = _nc_cache[key]

    chunks = _chunk_list(counts)

    # x packed: [128, KB1*T], chunk (t0, nb) occupies cols KB1*t0 .. KB1*(t0+nb)
    # laid out [k][t] (so the SBUF tile slice k*nb:(k+1)*nb is the k-th chunk)
    xs = np.asarray(x[order].T, dtype=_BF16)         # [H, T]
    xv = xs.reshape(_KB1, 128, T).transpose(1, 0, 2)  # [128, k, T]
    xP = np.empty((128, _KB1 * T), dtype=_BF16)
    for e_, t0, nb in chunks:
        xP[:, _KB1 * t0 : _KB1 * (t0 + nb)] = xv[:, :, t0 : t0 + nb].reshape(
            128, _KB1 * nb
        )

    wgu_bf = wgu_all.astype(_BF16)  # [E, H, 2F]
    wd_bf = wd_all.astype(_BF16)    # [E, F, H]

    in_maps = []
    for c in range(_NC):
        g = wgu_bf[:, :, c * FL : (c + 1) * FL]
        u = wgu_bf[:, :, F + c * FL : F + (c + 1) * FL]
        wgu_c = np.concatenate([g, u], axis=2)        # [E, H, F2L]
        # -> [E, p, k, f] flat as [E*128, KB1*F2L]
        wgu_pk = wgu_c.reshape(E, _KB1, 128, _F2L).transpose(0, 2, 1, 3)
        wgu_p = np.ascontiguousarray(wgu_pk).reshape(E * 128, _KB1 * _F2L)
        wd_c = wd_bf[:, c * FL : (c + 1) * FL, :]     # [E, FL, H]
        wd_pk = wd_c.reshape(E, _KB2, 128, H).transpose(0, 2, 1, 3)
        wd_p = np.ascontiguousarray(wd_pk).reshape(E * 128, _KB2 * H)
        in_maps.append({"xP": xP, "wgu": wgu_p, "wd": wd_p})

    res = run_bass_kernel_spmd(nc, in_maps, core_ids=list(range(_NC)))
    global last_run
    last_run = res

    # unpack chunk-packed yP [128, NH*T] and accumulate partials in fp32
    acc = np.zeros((H, T), np.float32)
    for c in range(_NC):
        yp = np.asarray(res.results[c]["yP"], dtype=np.float32)
        for e_, t0, nb in chunks:
            blk = yp[:, _NH * t0 : _NH * (t0 + nb)].reshape(128, _NH, nb)
            acc[:, t0 : t0 + nb] += blk.transpose(1, 0, 2).reshape(H, nb)
    out = np.empty((T, H), np.float32)
    out[order] = acc.T
    return out
